# revision 47
# baseline (speedup 1.0000x reference)
"""Trainium2 Bass kernel for MultiHeadAttention with RoPE.

Problem: B=2, L=2048, d_model=1024, 16 heads, d_k=64, fp32 in/out.

Sharding (8 cores): tensor-parallel over heads — core c owns heads
{2c, 2c+1}, i.e. a 128-wide slice of the projection output dims.  Every
core reads the full q/k/v activations (transposed + bf16 on host), its
own 128-row slice of Wq/Wk/Wv (pre-transposed; Wq/bq pre-scaled by
1/sqrt(dk)) and the matching 128 columns of Wo.  Each core computes its
heads' attention output and a partial d_model output projection; the
host sums the 8 partials and adds bo.

Per-core pipeline (bf16 matmuls, fp32 PSUM):
  1. QKV projections [128 pd, 1024 tok] halves; bias-add + bf16 evict on
     DVE; RoPE via partition-swap DMAs + 3 DVE ops (sign folded in sin
     table, 1/sqrt(dk) folded into Wq).
  2. V-heads transposed to [kt, dim] layout by ONE dma_start_transpose
     per (batch, head) into a 65-wide-stride "vaug" buffer whose 65th
     column is ones.
  3. Scores: per 128-kt tile, the two heads run CONCURRENTLY on the PE
     as K=64 row-tiles (tile_position (0,0) / (64,0) auto-derived).
  4. exp on ScalarE ([128, 1024] per kt tile covering both heads).
  5. ctx: lhsT = vaug [128 kt, 65] per head; row 64 accumulates the
     softmax denominator for free (M=65 stationary).
  6. normalize: DVE copy of cp, denominator row -> partition 0 via DMA,
     reciprocal + GpSimd broadcast + DVE muls; h1 ctx shifted to
     partitions 64:127 by a small DMA.
  7. out_proj [tok, 1024] = ctx (stationary) @ WoT slice.
Emission interleaves next-batch projections and out_proj matmuls into
the (ScalarE-bound) attention loops as "fillers" so the PE never idles
long and HAM stays at full clock.  All activation buffers are
per-batch tiles so filler writes never create false WAR dependencies
against the running attention.
"""

import collections
import numpy as np
import ml_dtypes

import concourse.bass as bass
import concourse.mybir as mybir
import concourse.tile as tile
from concourse import bacc
from concourse.bass_utils import run_bass_kernel_spmd

BF = mybir.dt.bfloat16
F32 = mybir.dt.float32
AF = mybir.ActivationFunctionType

NCORES = 8
B = 2
L = 2048
D = 1024          # d_model
H = 16            # heads
DK = 64           # head dim
HPC = H // NCORES  # heads per core = 2
PD = HPC * DK      # projection dims per core = 128
TOK = B * L        # 4096 tokens
P = 128
NKT = L // P       # 16 kt tiles per batch
NQB = 4            # 512-token q blocks per batch

ROPE_BASE = 10000.0


def build_nc(debug_dumps=False):
    """Build the single-core Bass program (SPMD: same program, per-core data)."""
    from contextlib import ExitStack

    nc = bacc.Bacc("TRN2", target_bir_lowering=False, debug=False)
    dbg = {}
    if debug_dumps:
        for nm, shp, dt in [
            ("dbg_qq", [P, L], BF), ("dbg_kk", [P, L], BF),
            ("dbg_vaug0", [P, NKT * P], BF), ("dbg_vaug1", [P, NKT * P], BF),
            ("dbg_exp", [P, 1024], BF), ("dbg_cps", [65, 1024], F32),
            ("dbg_rec", [1, 1024], F32), ("dbg_ctx", [P, L], BF),
        ]:
            dbg[nm] = nc.dram_tensor(nm, shp, dt, kind="ExternalOutput").ap()

    # ---- DRAM I/O ----
    # Host pre-arranges everything partition-contiguous so each DMA is 128
    # descriptors of big contiguous chunks (dispatch cost ~ descriptor count).
    # xH[p, (b, half, a, t)]: token t of half `half` of batch b, dim a*128+p.
    qH = nc.dram_tensor("qH", [P, B * 2 * 8 * 1024], BF, kind="ExternalInput").ap()
    kH = nc.dram_tensor("kH", [P, B * 2 * 8 * 1024], BF, kind="ExternalInput").ap()
    vH = nc.dram_tensor("vH", [P, B * 2 * 8 * 1024], BF, kind="ExternalInput").ap()
    # wH[p, (a, m)]: weight row a*128+p, output dim m.
    wqH = nc.dram_tensor("wqH", [P, 8 * P], BF, kind="ExternalInput").ap()
    wkH = nc.dram_tensor("wkH", [P, 8 * P], BF, kind="ExternalInput").ap()
    wvH = nc.dram_tensor("wvH", [P, 8 * P], BF, kind="ExternalInput").ap()
    woT = nc.dram_tensor("woT", [PD, D], BF, kind="ExternalInput").ap()
    bias_d = nc.dram_tensor("biases", [PD, 3], F32, kind="ExternalInput").ap()
    cos_d = nc.dram_tensor("cos_t", [P, L], BF, kind="ExternalInput").ap()
    sin_d = nc.dram_tensor("sin_t", [P, L], BF, kind="ExternalInput").ap()
    outp = nc.dram_tensor("outp", [TOK, D], BF, kind="ExternalOutput").ap()

    xT = {"q": qH, "k": kH, "v": vH}

    with tile.TileContext(nc) as tc, ExitStack() as ctx:
        const = ctx.enter_context(tc.tile_pool(name="const", bufs=1))
        persist = ctx.enter_context(tc.tile_pool(name="persist", bufs=1))
        stage = ctx.enter_context(tc.tile_pool(name="stage", bufs=5))
        raws = ctx.enter_context(tc.tile_pool(name="raws", bufs=2))
        rots = ctx.enter_context(tc.tile_pool(name="rots", bufs=2))
        expp = ctx.enter_context(tc.tile_pool(name="expp", bufs=3))
        outs = ctx.enter_context(tc.tile_pool(name="outs", bufs=3))
        smalls = ctx.enter_context(tc.tile_pool(name="smalls", bufs=2))
        h1p = ctx.enter_context(tc.tile_pool(name="h1p", bufs=2))
        # PSUM: scores 2 tiles x 2 banks + ctx 2 banks + proj/out 2 banks = 8
        scp = ctx.enter_context(tc.tile_pool(name="scp", bufs=2, space="PSUM"))
        cpp = ctx.enter_context(tc.tile_pool(name="cpp", bufs=1, space="PSUM"))
        pop = ctx.enter_context(tc.tile_pool(name="pop", bufs=1, space="PSUM"))

        # ---- constants (emitted in first-use order) ----
        # Head-phase bulk loads ride the SECOND hardware DMA queue (the
        # Activation engine, idle until the first exp) so gated loads never
        # block the rope/transpose/normalize chains on the sync queue.
        def load_w(name, w_d):
            w_sb = const.tile([P, 8 * P], BF, name=name)
            nc.scalar.dma_start(w_sb[:], w_d[:])
            return w_sb

        wk_sb = load_w("wk_sb", wkH)
        bias_sb3 = const.tile([P, 3], F32, name="bias_sb3")
        nc.scalar.dma_start(bias_sb3[:], bias_d[:])
        bq_sb = bias_sb3[:, 0:1]
        bk_sb = bias_sb3[:, 1:2]
        bv_sb = bias_sb3[:, 2:3]
        cos_sb = const.tile([P, L], BF)
        nc.scalar.dma_start(cos_sb[:], cos_d[:])
        sin_sb = const.tile([P, L], BF)
        nc.scalar.dma_start(sin_sb[:], sin_d[:])

        # per-(batch, token-half) persistent activations [128 dims, 1024 tok]
        # (split so scores never wait on the other half's rope chain)
        qq_h = [[persist.tile([P, 1024], BF, name=f"qq{b}_{hf}")
                 for hf in range(2)] for b in range(B)]
        kk_h = [[persist.tile([P, 1024], BF, name=f"kk{b}_{hf}")
                 for hf in range(2)] for b in range(B)]
        ctx_b = [persist.tile([P, L], BF, name=f"ctx{b}") for b in range(B)]
        # vaug[b][h]: 16 slots of [128 kt, 128]; cols 0:64 = v dims
        # (t-major: slot t partition p holds token t*128+p), cols 64:128 all
        # ones — so the ctx matmul replicates the softmax denominator into
        # cp rows 64:128 (no partition broadcast needed to normalize).
        vaug = [[persist.tile([P, NKT * P], BF, name=f"vaug_{b}_{h}")
                 for h in range(2)] for b in range(B)]
        for b in range(B):
            for h in range(2):
                va = vaug[b][h].rearrange("p (t u) -> p t u", u=P)
                nc.vector.memset(va[:, :, DK:P], 1.0)

        # ---------- filler machinery ----------
        fillers = collections.deque()

        def fill(budget):
            while fillers and budget > 0:
                cost, fn = fillers.popleft()
                fn()
                budget -= cost

        def flush():
            while fillers:
                fillers.popleft()[1]()

        # ---------- phase helpers ----------
        def load_half(which, b, half, eng=None):
            """One 2MB DMA, per-partition contiguous: a 1024-token half."""
            xt = stage.tile([P, 8 * 1024], BF, name="xstage", tag="stage")
            j = (b * 2 + half) * 8192
            (eng or nc.sync).dma_start(xt[:], xT[which][:, j:j + 8192])
            return xt

        def proj_units(which, b, w_sb, bias_sb, dst_sb=None, vh_cb=None,
                       preloaded=None, halves=(0, 1), shared=None):
            """Filler units projecting batch b (chosen 1024-token halves).

            dst_sb given -> rope into it (q/k).  vh_cb given -> v path:
            evict to a fresh vh tile, call vh_cb(vh_tile) when done.
            preloaded: stage tiles already loaded.  Pass the same `shared`
            dict when splitting one projection across two calls.
            """
            units = []
            if shared is None:
                shared = {}
            if preloaded is not None:
                shared[("x", 0)], shared[("x", 1)] = preloaded

            def alloc_pp(half):
                def go():
                    shared[("pp", half)] = pop.tile(
                        [P, 1024], F32, name="pp", tag="pp")
                return go

            def mm_kc(half, kc):
                def go():
                    xt = shared[("x", half)].rearrange("p (a t) -> p a t", a=8)
                    pp = shared[("pp", half)]
                    for nb in range(2):
                        nc.tensor.matmul(
                            pp[:, nb * 512:(nb + 1) * 512],
                            lhsT=w_sb[:, kc * P:(kc + 1) * P],
                            rhs=xt[:, kc, nb * 512:(nb + 1) * 512],
                            start=(kc == 0), stop=(kc == 7),
                        )
                return go

            def evict_rot(half):
                def go():
                    pp = shared[("pp", half)]
                    raw = raws.tile([P, 1024], BF, name="raw", tag="raw")
                    nc.vector.tensor_scalar_add(raw[:], pp[:], bias_sb[:])
                    rot = rots.tile([P, 1024], BF, name="rot", tag="rot")
                    for h in range(2):
                        r0 = h * DK
                        nc.sync.dma_start(rot[r0:r0 + 32, :],
                                          raw[r0 + 32:r0 + 64, :])
                        nc.sync.dma_start(rot[r0 + 32:r0 + 64, :],
                                          raw[r0:r0 + 32, :])
                    shared[("rr", half)] = (raw, rot)
                return go

            def rope_mul(half):
                def go():
                    raw, rot = shared[("rr", half)]
                    cs = slice(half * 1024, (half + 1) * 1024)
                    nc.vector.tensor_mul(raw[:], raw[:], cos_sb[:, cs])
                    nc.vector.tensor_mul(rot[:], rot[:], sin_sb[:, cs])
                    nc.vector.tensor_add(dst_sb[half][:], raw[:], rot[:])
                return go

            def evict_v(half):
                def go():
                    pp = shared[("pp", half)]
                    if "vh" not in shared:
                        shared["vh"] = raws.tile(
                            [P, L], BF, name="vhs", tag="vraw", bufs=2)
                    vh = shared["vh"]
                    nc.vector.tensor_scalar_add(
                        vh[:, half * 1024:(half + 1) * 1024], pp[:], bias_sb[:])
                    if half == 1:
                        vh_cb(vh)
                return go

            for half in halves:
                units.append((0, alloc_pp(half)))
                for kc in range(8):
                    units.append((2, mm_kc(half, kc)))
                if dst_sb is not None:
                    units.append((2, evict_rot(half)))
                    units.append((2, rope_mul(half)))
                else:
                    units.append((2, evict_v(half)))
            return units

        def vaug_transpose(b):
            # HW xbar transpose requires a contiguous destination; land in
            # per-head vt tiles then DVE-copy into the 65-stride vaug slots.
            def go(vh):
                for h in range(2):
                    vt = rots.tile([P, 1024], BF, name="vt", tag=f"vt{h}",
                                   bufs=2)
                    nc.sync.dma_start_transpose(
                        vt.rearrange("p (t u) -> p t u", u=64),
                        vh[h * DK:(h + 1) * DK, :])
                    va = vaug[b][h].rearrange(
                        "p (t u) -> p t u", u=P)[:, :, 0:DK]
                    nc.vector.tensor_copy(
                        va, vt.rearrange("p (t u) -> p t u", u=64))
            return go

        def attention(b, qb, budget=3):
            """512 q tokens; 16 kt tiles; 2 heads row-tiled on the PE."""
            qsl = slice((qb % 2) * 512, (qb % 2) * 512 + 512)
            qq = qq_h[b][qb // 2]
            q0 = qb * 512
            cp = cpp.tile([P, 1024], F32, name="cp", tag="cp")
            ex_prev = None
            for kt in range(NKT + 1):
                ex_cur = None
                if kt < NKT:
                    kk = kk_h[b][kt // 8]
                    ksl = slice((kt % 8) * P, (kt % 8) * P + P)
                    sc = scp.tile([P, 1024], F32, name="sc", tag="sc")
                    nc.tensor.matmul(
                        sc[:, 0:512],
                        lhsT=kk[0:DK, ksl],
                        rhs=qq[0:DK, qsl],
                        start=True, stop=True, skip_group_check=True)
                    nc.tensor.matmul(
                        sc[:, 512:1024],
                        lhsT=kk[DK:P, ksl],
                        rhs=qq[DK:P, qsl],
                        start=True, stop=True, skip_group_check=True)
                    ex_cur = expp.tile([P, 1024], BF, name="ex", tag="ex")
                    nc.scalar.activation(ex_cur[:], sc[:], AF.Exp)
                    if debug_dumps and b == 0 and qb == 0 and kt == 0:
                        nc.sync.dma_start(dbg["dbg_exp"][:], ex_cur[:])
                if kt >= 1:
                    c = kt - 1
                    for h in range(2):
                        nc.tensor.matmul(
                            cp[:, h * 512:(h + 1) * 512],
                            lhsT=vaug[b][h][:, c * P:(c + 1) * P],
                            rhs=ex_prev[:, h * 512:(h + 1) * 512],
                            start=(c == 0), stop=(c == NKT - 1),
                            skip_group_check=True)
                ex_prev = ex_cur
                fill(budget)
            # normalize: cp rows 64:128 all hold the denominator; copy cp
            # out (frees the psum), DMA the denominator rows to partition 0,
            # reciprocal, scale both heads.
            cps = smalls.tile([P, 1024], F32, name="cps", tag="cps")
            nc.vector.tensor_copy(cps[:], cp[:])
            den = smalls.tile([DK, 1024], F32, name="den", tag="den", bufs=1)
            nc.sync.dma_start(den[:], cps[DK:P, :])
            rec = smalls.tile([DK, 1024], F32, name="rec", tag="rec", bufs=1)
            nc.vector.reciprocal_approx_fast(rec[:], den[:])
            if debug_dumps and b == 0 and qb == 0:
                nc.sync.dma_start(dbg["dbg_cps"][:], cps[0:65, :])
                nc.sync.dma_start(dbg["dbg_rec"][:], rec[0:1, :])
            nc.vector.tensor_mul(
                ctx_b[b][0:DK, q0:q0 + 512], cps[0:DK, 0:512], rec[:, 0:512])
            h1s = h1p.tile([DK, 512], BF, name="h1s", tag="h1s")
            nc.vector.tensor_mul(h1s[:], cps[0:DK, 512:1024], rec[:, 512:1024])
            nc.sync.dma_start(ctx_b[b][DK:P, q0:q0 + 512], h1s[:])

        def out_units(b, qb, tail=False):
            units = []

            def po_unit(tb):
                def go():
                    t0 = qb * 512 + tb * P
                    # in the end-of-kernel tail, alternate PSUM pools so
                    # consecutive units pipeline instead of serializing on
                    # the single pop buffer (scores pool is idle by then)
                    if tail and tb % 2:
                        po = scp.tile([P, D], F32, name="sc", tag="sc")
                    else:
                        po = pop.tile([P, D], F32, name="po", tag="pp")
                    for nb in range(2):
                        nc.tensor.matmul(
                            po[:, nb * 512:(nb + 1) * 512],
                            lhsT=ctx_b[b][:, t0:t0 + P],
                            rhs=wo_sb[:, nb * 512:(nb + 1) * 512],
                            start=True, stop=True, skip_group_check=True)
                    ob = outs.tile([P, D], BF, name="ob", tag="ob")
                    nc.vector.tensor_copy(ob[:], po[:])
                    nc.sync.dma_start(outp[b * L + t0:b * L + t0 + P, :], ob[:])
                return go

            for tb in range(4):
                units.append((2, po_unit(tb)))
            return units

        def load_bar(tile):
            """Tiny DMA reading `tile`: holds its queue until tile's load
            transfer completes, serializing big transfers so they finish
            in consumption order instead of round-robin sharing."""
            dm = smalls.tile([1, 2], BF, name="bar", tag="bar", bufs=2)
            nc.scalar.dma_start(dm[:], tile[0:1, 0:2])

        # ---------- program ----------
        # Batch-0 inputs prefetched in consumption-criticality order and
        # SERIALIZED via load barriers: v first (the vaug chain gates the
        # first ctx matmul), then the h0 halves of k and q (which gate the
        # first scores), then k-h1 / q-h1 for the filler projections.
        wv_sb = load_w("wv_sb", wvH)
        wq_sb = load_w("wq_sb", wqH)
        v0a = load_half("v", 0, 0, eng=nc.scalar)
        v0b = load_half("v", 0, 1, eng=nc.scalar)
        v0_x = (v0a, v0b)
        load_bar(v0a)
        k0a = load_half("k", 0, 0, eng=nc.scalar)
        wo_sb = const.tile([P, D], BF)
        nc.scalar.dma_start(wo_sb[:], woT[:])
        load_bar(v0b)
        q0a = load_half("q", 0, 0, eng=nc.scalar)
        load_bar(k0a)
        k0b = load_half("k", 0, 1, eng=nc.scalar)
        load_bar(q0a)
        q0b = load_half("q", 0, 1, eng=nc.scalar)
        k0_x = (k0a, k0b)
        q0_x = (q0a, q0b)

        for _, fn in proj_units("v", 0, wv_sb, bv_sb, vh_cb=vaug_transpose(0),
                                preloaded=v0_x):
            fn()
        ksh, qsh = {}, {}
        for _, fn in proj_units("k", 0, wk_sb, bk_sb, dst_sb=kk_h[0],
                                preloaded=k0_x, halves=(0,), shared=ksh):
            fn()
        for _, fn in proj_units("q", 0, wq_sb, bq_sb, dst_sb=qq_h[0],
                                preloaded=q0_x, halves=(0,), shared=qsh):
            fn()
        # second halves become the first attention fillers
        fillers.extend(proj_units("k", 0, wk_sb, bk_sb, dst_sb=kk_h[0],
                                  halves=(1,), shared=ksh))
        fillers.extend(proj_units("q", 0, wq_sb, bq_sb, dst_sb=qq_h[0],
                                  halves=(1,), shared=qsh))
        # batch-1 k prefetch rides the tail of the batch-0 load stream
        xk1 = (load_half("k", 1, 0), load_half("k", 1, 1))

        # batch 0 attention; feed batch-1 projections + batch-0 out_proj
        # into the scalar-bound loop as fillers (batch-1 input DMAs are
        # dispatched inline at block boundaries for transfer lead time).
        for qb in range(NQB):
            attention(0, qb)
            fillers.extend(out_units(0, qb))
            if qb == 0:
                xv1 = (load_half("v", 1, 0), load_half("v", 1, 1))
                fillers.extend(proj_units("k", 1, wk_sb, bk_sb,
                                          dst_sb=kk_h[1], preloaded=xk1))
                fillers.extend(proj_units("v", 1, wv_sb, bv_sb,
                                          vh_cb=vaug_transpose(1),
                                          preloaded=xv1))
            elif qb == 1:
                xq1 = (load_half("q", 1, 0), load_half("q", 1, 1))
                fillers.extend(proj_units("q", 1, wq_sb, bq_sb,
                                          dst_sb=qq_h[1], preloaded=xq1))
        flush()  # b1 attention depends on b1 projections: drain first

        for qb in range(NQB):
            attention(1, qb)
            fillers.extend(out_units(1, qb, tail=(qb == NQB - 1)))
        flush()

        if debug_dumps:
            for hf in range(2):
                nc.sync.dma_start(
                    dbg["dbg_qq"][:, hf * 1024:(hf + 1) * 1024], qq_h[0][hf][:])
                nc.sync.dma_start(
                    dbg["dbg_kk"][:, hf * 1024:(hf + 1) * 1024], kk_h[0][hf][:])
            nc.sync.dma_start(dbg["dbg_vaug0"][:], vaug[0][0][:])
            nc.sync.dma_start(dbg["dbg_vaug1"][:], vaug[0][1][:])
            nc.sync.dma_start(dbg["dbg_ctx"][:], ctx_b[0][:])

    return nc


def _rope_tables():
    """Host-built RoPE tables [d, t], 2 heads stacked, sign-folded sin."""
    inv_freq = 1.0 / (ROPE_BASE ** (np.arange(0, DK, 2, dtype=np.float64) / DK))
    t = np.arange(L, dtype=np.float64)
    ang = np.outer(t, inv_freq)               # [L, 32]
    emb = np.concatenate([ang, ang], axis=1)  # [L, 64]
    cos = np.cos(emb).T.astype(np.float32)    # [64, L]
    sin = np.sin(emb).T.astype(np.float32)
    sin_folded = sin.copy()
    sin_folded[:32] *= -1.0
    bf = ml_dtypes.bfloat16
    cos2 = np.concatenate([cos, cos], axis=0)                # [128, L]
    sin2 = np.concatenate([sin_folded, sin_folded], axis=0)  # [128, L]
    return cos2.astype(bf), sin2.astype(bf)


def _xh(x):
    """[B, L, D] -> [128, B*2*8*1024]: xH[p, (b, hf, a, t)] = x[b, hf*1024+t,
    a*128+p] — every (b, half) load is per-partition contiguous."""
    bf = ml_dtypes.bfloat16
    xr = np.asarray(x).reshape(B, 2, 1024, 8, P).transpose(4, 0, 1, 3, 2)
    return np.ascontiguousarray(xr.reshape(P, B * 2 * 8 * 1024)).astype(bf)


def _wh(w_slice):
    """[PD, D] weight slice -> [128, 8*128]: wH[p, (a, m)] = W.T[a*128+p, m]."""
    bf = ml_dtypes.bfloat16
    wr = np.ascontiguousarray(w_slice.T).reshape(8, P, PD).transpose(1, 0, 2)
    return np.ascontiguousarray(wr.reshape(P, 8 * PD)).astype(bf)


def host_in_maps(q, k, v, Wq, bq, Wk, bk, Wv, bv, Wo):
    """Per-core input maps (the 1/sqrt(dk) scale is folded into Wq/bq)."""
    bf = ml_dtypes.bfloat16
    qh, kh, vh = _xh(q), _xh(k), _xh(v)
    cos_t, sin_t = _rope_tables()
    scale = 1.0 / np.sqrt(DK)
    in_maps = []
    for c in range(NCORES):
        hs = slice(c * PD, (c + 1) * PD)
        biases = np.stack([
            np.asarray(bq)[hs] * scale, np.asarray(bk)[hs],
            np.asarray(bv)[hs]], axis=1).astype(np.float32)
        in_maps.append({
            "qH": qh, "kH": kh, "vH": vh,
            "wqH": _wh(np.asarray(Wq)[hs, :] * scale),
            "wkH": _wh(np.asarray(Wk)[hs, :]),
            "wvH": _wh(np.asarray(Wv)[hs, :]),
            "woT": np.ascontiguousarray(np.asarray(Wo)[:, hs].T).astype(bf),
            "biases": biases,
            "cos_t": cos_t, "sin_t": sin_t,
        })
    return in_maps


_NC_CACHE = {}


def _get_nc():
    if "nc" not in _NC_CACHE:
        nc = build_nc()
        nc.finalize()
        _NC_CACHE["nc"] = nc
    return _NC_CACHE["nc"]


def kernel(q, k, v, Wq, bq, Wk, bk, Wv, bv, Wo, bo):
    assert q.shape == (B, L, D) and k.shape == (B, L, D) and v.shape == (B, L, D)
    in_maps = host_in_maps(q, k, v, Wq, bq, Wk, bk, Wv, bv, Wo)
    nc = _get_nc()
    res = run_bass_kernel_spmd(nc, in_maps, list(range(NCORES)))
    out = np.zeros((TOK, D), np.float64)
    for r in res.results:
        out += r["outp"].astype(np.float64)
    out += np.asarray(bo, np.float64)[None, :]
    return out.astype(np.float32).reshape(B, L, D)


# revision 52
# speedup vs baseline: 1.1786x; 1.1786x over previous
"""Trainium2 Bass kernel for MultiHeadAttention with RoPE.

Problem: B=2, L=2048, d_model=1024, 16 heads, d_k=64, fp32 in/out.

Sharding (8 cores): tensor-parallel over heads — core c owns heads
{2c, 2c+1}, i.e. a 128-wide slice of the projection output dims.  Every
core reads the full q/k/v activations (transposed + bf16 on host), its
own 128-row slice of Wq/Wk/Wv (pre-transposed; Wq/bq pre-scaled by
1/sqrt(dk)) and the matching 128 columns of Wo.  Each core computes its
heads' attention output and a partial d_model output projection; the
host sums the 8 partials and adds bo.

Per-core pipeline (bf16 matmuls, fp32 PSUM):
  1. QKV projections [128 pd, 1024 tok] halves; bias-add + bf16 evict on
     DVE; RoPE via partition-swap DMAs + 3 DVE ops (sign folded in sin
     table, 1/sqrt(dk) folded into Wq).
  2. V-heads transposed to [kt, dim] layout by ONE dma_start_transpose
     per (batch, head) into a 65-wide-stride "vaug" buffer whose 65th
     column is ones.
  3. Scores: per 128-kt tile, the two heads run CONCURRENTLY on the PE
     as K=64 row-tiles (tile_position (0,0) / (64,0) auto-derived).
  4. exp on ScalarE ([128, 1024] per kt tile covering both heads).
  5. ctx: lhsT = vaug [128 kt, 65] per head; row 64 accumulates the
     softmax denominator for free (M=65 stationary).
  6. normalize: DVE copy of cp, denominator row -> partition 0 via DMA,
     reciprocal + GpSimd broadcast + DVE muls; h1 ctx shifted to
     partitions 64:127 by a small DMA.
  7. out_proj [tok, 1024] = ctx (stationary) @ WoT slice.
Emission interleaves next-batch projections and out_proj matmuls into
the (ScalarE-bound) attention loops as "fillers" so the PE never idles
long and HAM stays at full clock.  All activation buffers are
per-batch tiles so filler writes never create false WAR dependencies
against the running attention.
"""

import collections
import numpy as np
import ml_dtypes

import concourse.bass as bass
import concourse.mybir as mybir
import concourse.tile as tile
from concourse import bacc
from concourse.bass_utils import run_bass_kernel_spmd

BF = mybir.dt.bfloat16
F32 = mybir.dt.float32
AF = mybir.ActivationFunctionType

NCORES = 8
B = 2
L = 2048
D = 1024          # d_model
H = 16            # heads
DK = 64           # head dim
HPC = H // NCORES  # heads per core = 2
PD = HPC * DK      # projection dims per core = 128
TOK = B * L        # 4096 tokens
P = 128
NKT = L // P       # 16 kt tiles per batch
NQB = 4            # 512-token q blocks per batch

ROPE_BASE = 10000.0


def build_nc(debug_dumps=False):
    """Build the single-core Bass program (SPMD: same program, per-core data)."""
    from contextlib import ExitStack

    nc = bacc.Bacc("TRN2", target_bir_lowering=False, debug=False)
    dbg = {}
    if debug_dumps:
        for nm, shp, dt in [
            ("dbg_qq", [P, L], BF), ("dbg_kk", [P, L], BF),
            ("dbg_vaug0", [P, NKT * P], BF), ("dbg_vaug1", [P, NKT * P], BF),
            ("dbg_exp", [P, 1024], BF), ("dbg_cps", [65, 1024], F32),
            ("dbg_rec", [1, 1024], F32), ("dbg_ctx", [P, L], BF),
        ]:
            dbg[nm] = nc.dram_tensor(nm, shp, dt, kind="ExternalOutput").ap()

    # ---- DRAM I/O ----
    # Host pre-arranges everything partition-contiguous so each DMA is 128
    # descriptors of big contiguous chunks (dispatch cost ~ descriptor count).
    # xH[p, (b, half, a, t)]: token t of half `half` of batch b, dim a*128+p.
    qH = nc.dram_tensor("qH", [P, B * 2 * 8 * 1024], BF, kind="ExternalInput").ap()
    kH = nc.dram_tensor("kH", [P, B * 2 * 8 * 1024], BF, kind="ExternalInput").ap()
    vH = nc.dram_tensor("vH", [P, B * 2 * 8 * 1024], BF, kind="ExternalInput").ap()
    # wH[p, (a, m)]: weight row a*128+p, output dim m.
    wqH = nc.dram_tensor("wqH", [P, 8 * P], BF, kind="ExternalInput").ap()
    wkH = nc.dram_tensor("wkH", [P, 8 * P], BF, kind="ExternalInput").ap()
    wvH = nc.dram_tensor("wvH", [P, 8 * P], BF, kind="ExternalInput").ap()
    woT = nc.dram_tensor("woT", [PD, D], BF, kind="ExternalInput").ap()
    bias_d = nc.dram_tensor("biases", [PD, 3], F32, kind="ExternalInput").ap()
    cos_d = nc.dram_tensor("cos_t", [P, L], BF, kind="ExternalInput").ap()
    sin_d = nc.dram_tensor("sin_t", [P, L], BF, kind="ExternalInput").ap()
    outp = nc.dram_tensor("outp", [TOK, D], BF, kind="ExternalOutput").ap()

    xT = {"q": qH, "k": kH, "v": vH}

    with tile.TileContext(nc) as tc, ExitStack() as ctx:
        const = ctx.enter_context(tc.tile_pool(name="const", bufs=1))
        persist = ctx.enter_context(tc.tile_pool(name="persist", bufs=1))
        stage = ctx.enter_context(tc.tile_pool(name="stage", bufs=5))
        raws = ctx.enter_context(tc.tile_pool(name="raws", bufs=2))
        rots = ctx.enter_context(tc.tile_pool(name="rots", bufs=2))
        expp = ctx.enter_context(tc.tile_pool(name="expp", bufs=3))
        outs = ctx.enter_context(tc.tile_pool(name="outs", bufs=3))
        smalls = ctx.enter_context(tc.tile_pool(name="smalls", bufs=2))
        h1p = ctx.enter_context(tc.tile_pool(name="h1p", bufs=2))
        # PSUM: scores 2 tiles x 2 banks + ctx 2 banks + proj/out 2 banks = 8
        scp = ctx.enter_context(tc.tile_pool(name="scp", bufs=2, space="PSUM"))
        cpp = ctx.enter_context(tc.tile_pool(name="cpp", bufs=1, space="PSUM"))
        pop = ctx.enter_context(tc.tile_pool(name="pop", bufs=1, space="PSUM"))

        # ---- constants (declared here, loads emitted inside the barrier
        # chain below in first-use order) ----
        def load_w(name, w_d):
            w_sb = const.tile([P, 8 * P], BF, name=name)
            nc.sync.dma_start(w_sb[:], w_d[:])
            return w_sb

        bias_sb3 = const.tile([P, 3], F32, name="bias_sb3")
        bq_sb = bias_sb3[:, 0:1]
        bk_sb = bias_sb3[:, 1:2]
        bv_sb = bias_sb3[:, 2:3]
        cos_sb = const.tile([P, L], BF, name="cos_sb")
        sin_sb = const.tile([P, L], BF, name="sin_sb")

        # per-(batch, token-half) persistent activations [128 dims, 1024 tok]
        # (split so scores never wait on the other half's rope chain)
        qq_h = [[persist.tile([P, 1024], BF, name=f"qq{b}_{hf}")
                 for hf in range(2)] for b in range(B)]
        kk_h = [[persist.tile([P, 1024], BF, name=f"kk{b}_{hf}")
                 for hf in range(2)] for b in range(B)]
        ctx_b = [persist.tile([P, L], BF, name=f"ctx{b}") for b in range(B)]
        # vaug[b][h]: 16 slots of [128 kt, 128]; cols 0:64 = v dims
        # (t-major: slot t partition p holds token t*128+p), cols 64:128 all
        # ones — so the ctx matmul replicates the softmax denominator into
        # cp rows 64:128 (no partition broadcast needed to normalize).
        vaug = [[persist.tile([P, NKT * P], BF, name=f"vaug_{b}_{h}")
                 for h in range(2)] for b in range(B)]
        for b in range(B):
            for h in range(2):
                va = vaug[b][h].rearrange("p (t u) -> p t u", u=P)
                nc.vector.memset(va[:, :, DK:P], 1.0)

        # ---------- filler machinery ----------
        fillers = collections.deque()

        def fill(budget):
            while fillers and budget > 0:
                cost, fn = fillers.popleft()
                fn()
                budget -= cost

        def flush():
            while fillers:
                fillers.popleft()[1]()

        # ---------- phase helpers ----------
        def load_half(which, b, half, eng=None):
            """One 2MB DMA, per-partition contiguous: a 1024-token half."""
            xt = stage.tile([P, 8 * 1024], BF, name="xstage", tag="stage")
            j = (b * 2 + half) * 8192
            (eng or nc.sync).dma_start(xt[:], xT[which][:, j:j + 8192])
            return xt

        def proj_units(which, b, w_sb, bias_sb, dst_sb=None, vh_cb=None,
                       preloaded=None, halves=(0, 1), shared=None):
            """Filler units projecting batch b (chosen 1024-token halves).

            dst_sb given -> rope into it (q/k).  vh_cb given -> v path:
            evict to a fresh vh tile, call vh_cb(vh_tile) when done.
            preloaded: stage tiles already loaded.  Pass the same `shared`
            dict when splitting one projection across two calls.
            """
            units = []
            if shared is None:
                shared = {}
            if preloaded is not None:
                shared[("x", 0)], shared[("x", 1)] = preloaded

            def alloc_pp(half):
                def go():
                    shared[("pp", half)] = pop.tile(
                        [P, 1024], F32, name="pp", tag="pp")
                return go

            def mm_kc(half, kc):
                def go():
                    xt = shared[("x", half)].rearrange("p (a t) -> p a t", a=8)
                    pp = shared[("pp", half)]
                    for nb in range(2):
                        nc.tensor.matmul(
                            pp[:, nb * 512:(nb + 1) * 512],
                            lhsT=w_sb[:, kc * P:(kc + 1) * P],
                            rhs=xt[:, kc, nb * 512:(nb + 1) * 512],
                            start=(kc == 0), stop=(kc == 7),
                        )
                return go

            def evict_rot(half):
                def go():
                    pp = shared[("pp", half)]
                    raw = raws.tile([P, 1024], BF, name="raw", tag="raw")
                    nc.vector.tensor_scalar_add(raw[:], pp[:], bias_sb[:])
                    rot = rots.tile([P, 1024], BF, name="rot", tag="rot")
                    for h in range(2):
                        r0 = h * DK
                        nc.sync.dma_start(rot[r0:r0 + 32, :],
                                          raw[r0 + 32:r0 + 64, :])
                        nc.sync.dma_start(rot[r0 + 32:r0 + 64, :],
                                          raw[r0:r0 + 32, :])
                    shared[("rr", half)] = (raw, rot)
                return go

            def rope_mul(half):
                def go():
                    raw, rot = shared[("rr", half)]
                    cs = slice(half * 1024, (half + 1) * 1024)
                    nc.vector.tensor_mul(raw[:], raw[:], cos_sb[:, cs])
                    nc.vector.tensor_mul(rot[:], rot[:], sin_sb[:, cs])
                    nc.vector.tensor_add(dst_sb[half][:], raw[:], rot[:])
                return go

            def evict_v(half):
                def go():
                    pp = shared[("pp", half)]
                    if "vh" not in shared:
                        shared["vh"] = raws.tile(
                            [P, L], BF, name="vhs", tag="vraw", bufs=2)
                    vh = shared["vh"]
                    nc.vector.tensor_scalar_add(
                        vh[:, half * 1024:(half + 1) * 1024], pp[:], bias_sb[:])
                    if half == 1:
                        vh_cb(vh)
                return go

            for half in halves:
                units.append((0, alloc_pp(half)))
                for kc in range(8):
                    units.append((2, mm_kc(half, kc)))
                if dst_sb is not None:
                    units.append((2, evict_rot(half)))
                    units.append((2, rope_mul(half)))
                else:
                    units.append((2, evict_v(half)))
            return units

        def vaug_transpose(b):
            # HW xbar transpose requires a contiguous destination; land in
            # per-head vt tiles then DVE-copy into the vaug slots.  Batch 0
            # uses the (pre-exp idle) scalar hwdge queue so the transpose's
            # queue-drain never blocks the rope DMAs behind it on sync.
            eng = nc.scalar if b == 0 else nc.sync
            def go(vh):
                for h in range(2):
                    vt = rots.tile([P, 1024], BF, name="vt", tag=f"vt{h}",
                                   bufs=2)
                    eng.dma_start_transpose(
                        vt.rearrange("p (t u) -> p t u", u=64),
                        vh[h * DK:(h + 1) * DK, :])
                    va = vaug[b][h].rearrange(
                        "p (t u) -> p t u", u=P)[:, :, 0:DK]
                    nc.vector.tensor_copy(
                        va, vt.rearrange("p (t u) -> p t u", u=64))
            return go

        def attention(b, qb, budget=3):
            """512 q tokens; 16 kt tiles; 2 heads row-tiled on the PE."""
            qsl = slice((qb % 2) * 512, (qb % 2) * 512 + 512)
            qq = qq_h[b][qb // 2]
            q0 = qb * 512
            cp = cpp.tile([P, 1024], F32, name="cp", tag="cp")
            ex_prev = None
            for kt in range(NKT + 1):
                ex_cur = None
                if kt < NKT:
                    kk = kk_h[b][kt // 8]
                    ksl = slice((kt % 8) * P, (kt % 8) * P + P)
                    sc = scp.tile([P, 1024], F32, name="sc", tag="sc")
                    nc.tensor.matmul(
                        sc[:, 0:512],
                        lhsT=kk[0:DK, ksl],
                        rhs=qq[0:DK, qsl],
                        start=True, stop=True, skip_group_check=True)
                    nc.tensor.matmul(
                        sc[:, 512:1024],
                        lhsT=kk[DK:P, ksl],
                        rhs=qq[DK:P, qsl],
                        start=True, stop=True, skip_group_check=True)
                    ex_cur = expp.tile([P, 1024], BF, name="ex", tag="ex")
                    nc.scalar.activation(ex_cur[:], sc[:], AF.Exp)
                    if debug_dumps and b == 0 and qb == 0 and kt == 0:
                        nc.sync.dma_start(dbg["dbg_exp"][:], ex_cur[:])
                if kt >= 1:
                    c = kt - 1
                    for h in range(2):
                        nc.tensor.matmul(
                            cp[:, h * 512:(h + 1) * 512],
                            lhsT=vaug[b][h][:, c * P:(c + 1) * P],
                            rhs=ex_prev[:, h * 512:(h + 1) * 512],
                            start=(c == 0), stop=(c == NKT - 1),
                            skip_group_check=True)
                ex_prev = ex_cur
                fill(budget)
            # normalize: cp rows 64:128 all hold the denominator; copy cp
            # out (frees the psum), DMA the denominator rows to partition 0,
            # reciprocal, scale both heads.
            cps = smalls.tile([P, 1024], F32, name="cps", tag="cps")
            nc.vector.tensor_copy(cps[:], cp[:])
            den = smalls.tile([DK, 1024], F32, name="den", tag="den", bufs=1)
            nc.sync.dma_start(den[:], cps[DK:P, :])
            rec = smalls.tile([DK, 1024], F32, name="rec", tag="rec", bufs=1)
            nc.vector.reciprocal_approx_fast(rec[:], den[:])
            if debug_dumps and b == 0 and qb == 0:
                nc.sync.dma_start(dbg["dbg_cps"][:], cps[0:65, :])
                nc.sync.dma_start(dbg["dbg_rec"][:], rec[0:1, :])
            nc.vector.tensor_mul(
                ctx_b[b][0:DK, q0:q0 + 512], cps[0:DK, 0:512], rec[:, 0:512])
            h1s = h1p.tile([DK, 512], BF, name="h1s", tag="h1s")
            nc.vector.tensor_mul(h1s[:], cps[0:DK, 512:1024], rec[:, 512:1024])
            nc.sync.dma_start(ctx_b[b][DK:P, q0:q0 + 512], h1s[:])

        def out_units(b, qb, tail=False):
            units = []

            def po_unit(tb):
                def go():
                    t0 = qb * 512 + tb * P
                    # in the end-of-kernel tail, alternate PSUM pools so
                    # consecutive units pipeline instead of serializing on
                    # the single pop buffer (scores pool is idle by then)
                    if tail and tb % 2:
                        po = scp.tile([P, D], F32, name="sc", tag="sc")
                    else:
                        po = pop.tile([P, D], F32, name="po", tag="pp")
                    for nb in range(2):
                        nc.tensor.matmul(
                            po[:, nb * 512:(nb + 1) * 512],
                            lhsT=ctx_b[b][:, t0:t0 + P],
                            rhs=wo_sb[:, nb * 512:(nb + 1) * 512],
                            start=True, stop=True, skip_group_check=True)
                    ob = outs.tile([P, D], BF, name="ob", tag="ob")
                    nc.vector.tensor_copy(ob[:], po[:])
                    nc.sync.dma_start(outp[b * L + t0:b * L + t0 + P, :], ob[:])
                return go

            for tb in range(4):
                units.append((2, po_unit(tb)))
            return units

        def load_bar(tile):
            """Tiny DMA reading `tile`: holds its queue until tile's load
            transfer completes, serializing big transfers so they finish
            in consumption order instead of round-robin sharing."""
            dm = smalls.tile([1, 2], BF, name="bar", tag="bar", bufs=2)
            nc.sync.dma_start(dm[:], tile[0:1, 0:2])

        # ---------- program ----------
        # Batch-0 inputs prefetched in consumption-criticality order and
        # SERIALIZED via load barriers: v first (the vaug chain gates the
        # first ctx matmul), then the h0 halves of k and q (which gate the
        # first scores), then k-h1 / q-h1 for the filler projections.
        # v first (gates the first ctx), then wv; k-h0/q-h0 (gate the first
        # scores) with their weights and the rope tables slotted into the
        # barrier gaps; k-h1/q-h1 last.
        v0a = load_half("v", 0, 0)
        v0b = load_half("v", 0, 1)
        v0_x = (v0a, v0b)
        load_bar(v0a)
        wv_sb = load_w("wv_sb", wvH)
        nc.sync.dma_start(bias_sb3[:], bias_d[:])
        k0a = load_half("k", 0, 0)
        load_bar(v0b)
        wk_sb = load_w("wk_sb", wkH)
        q0a = load_half("q", 0, 0)
        wq_sb = load_w("wq_sb", wqH)
        load_bar(k0a)
        nc.sync.dma_start(cos_sb[:], cos_d[:])
        nc.sync.dma_start(sin_sb[:], sin_d[:])
        k0b = load_half("k", 0, 1)
        load_bar(q0a)
        q0b = load_half("q", 0, 1)
        wo_sb = const.tile([P, D], BF)
        nc.sync.dma_start(wo_sb[:], woT[:])
        k0_x = (k0a, k0b)
        q0_x = (q0a, q0b)

        for _, fn in proj_units("v", 0, wv_sb, bv_sb, vh_cb=vaug_transpose(0),
                                preloaded=v0_x):
            fn()
        ksh, qsh = {}, {}
        for _, fn in proj_units("k", 0, wk_sb, bk_sb, dst_sb=kk_h[0],
                                preloaded=k0_x, halves=(0,), shared=ksh):
            fn()
        for _, fn in proj_units("q", 0, wq_sb, bq_sb, dst_sb=qq_h[0],
                                preloaded=q0_x, halves=(0,), shared=qsh):
            fn()
        # second halves become the first attention fillers
        fillers.extend(proj_units("k", 0, wk_sb, bk_sb, dst_sb=kk_h[0],
                                  halves=(1,), shared=ksh))
        fillers.extend(proj_units("q", 0, wq_sb, bq_sb, dst_sb=qq_h[0],
                                  halves=(1,), shared=qsh))
        # batch-1 k prefetch rides the tail of the batch-0 load stream
        xk1 = (load_half("k", 1, 0), load_half("k", 1, 1))

        # batch 0 attention; feed batch-1 projections + batch-0 out_proj
        # into the scalar-bound loop as fillers (batch-1 input DMAs are
        # dispatched inline at block boundaries for transfer lead time).
        for qb in range(NQB):
            attention(0, qb)
            fillers.extend(out_units(0, qb))
            if qb == 0:
                xv1 = (load_half("v", 1, 0), load_half("v", 1, 1))
                fillers.extend(proj_units("k", 1, wk_sb, bk_sb,
                                          dst_sb=kk_h[1], preloaded=xk1))
                fillers.extend(proj_units("v", 1, wv_sb, bv_sb,
                                          vh_cb=vaug_transpose(1),
                                          preloaded=xv1))
            elif qb == 1:
                xq1 = (load_half("q", 1, 0), load_half("q", 1, 1))
                fillers.extend(proj_units("q", 1, wq_sb, bq_sb,
                                          dst_sb=qq_h[1], preloaded=xq1))
        flush()  # b1 attention depends on b1 projections: drain first

        for qb in range(NQB):
            attention(1, qb)
            fillers.extend(out_units(1, qb, tail=(qb == NQB - 1)))
        flush()

        if debug_dumps:
            for hf in range(2):
                nc.sync.dma_start(
                    dbg["dbg_qq"][:, hf * 1024:(hf + 1) * 1024], qq_h[0][hf][:])
                nc.sync.dma_start(
                    dbg["dbg_kk"][:, hf * 1024:(hf + 1) * 1024], kk_h[0][hf][:])
            nc.sync.dma_start(dbg["dbg_vaug0"][:], vaug[0][0][:])
            nc.sync.dma_start(dbg["dbg_vaug1"][:], vaug[0][1][:])
            nc.sync.dma_start(dbg["dbg_ctx"][:], ctx_b[0][:])

    return nc


def _rope_tables():
    """Host-built RoPE tables [d, t], 2 heads stacked, sign-folded sin."""
    inv_freq = 1.0 / (ROPE_BASE ** (np.arange(0, DK, 2, dtype=np.float64) / DK))
    t = np.arange(L, dtype=np.float64)
    ang = np.outer(t, inv_freq)               # [L, 32]
    emb = np.concatenate([ang, ang], axis=1)  # [L, 64]
    cos = np.cos(emb).T.astype(np.float32)    # [64, L]
    sin = np.sin(emb).T.astype(np.float32)
    sin_folded = sin.copy()
    sin_folded[:32] *= -1.0
    bf = ml_dtypes.bfloat16
    cos2 = np.concatenate([cos, cos], axis=0)                # [128, L]
    sin2 = np.concatenate([sin_folded, sin_folded], axis=0)  # [128, L]
    return cos2.astype(bf), sin2.astype(bf)


def _xh(x):
    """[B, L, D] -> [128, B*2*8*1024]: xH[p, (b, hf, a, t)] = x[b, hf*1024+t,
    a*128+p] — every (b, half) load is per-partition contiguous."""
    bf = ml_dtypes.bfloat16
    xr = np.asarray(x).reshape(B, 2, 1024, 8, P).transpose(4, 0, 1, 3, 2)
    return np.ascontiguousarray(xr.reshape(P, B * 2 * 8 * 1024)).astype(bf)


def _wh(w_slice):
    """[PD, D] weight slice -> [128, 8*128]: wH[p, (a, m)] = W.T[a*128+p, m]."""
    bf = ml_dtypes.bfloat16
    wr = np.ascontiguousarray(w_slice.T).reshape(8, P, PD).transpose(1, 0, 2)
    return np.ascontiguousarray(wr.reshape(P, 8 * PD)).astype(bf)


def host_in_maps(q, k, v, Wq, bq, Wk, bk, Wv, bv, Wo):
    """Per-core input maps (the 1/sqrt(dk) scale is folded into Wq/bq)."""
    bf = ml_dtypes.bfloat16
    qh, kh, vh = _xh(q), _xh(k), _xh(v)
    cos_t, sin_t = _rope_tables()
    scale = 1.0 / np.sqrt(DK)
    in_maps = []
    for c in range(NCORES):
        hs = slice(c * PD, (c + 1) * PD)
        biases = np.stack([
            np.asarray(bq)[hs] * scale, np.asarray(bk)[hs],
            np.asarray(bv)[hs]], axis=1).astype(np.float32)
        in_maps.append({
            "qH": qh, "kH": kh, "vH": vh,
            "wqH": _wh(np.asarray(Wq)[hs, :] * scale),
            "wkH": _wh(np.asarray(Wk)[hs, :]),
            "wvH": _wh(np.asarray(Wv)[hs, :]),
            "woT": np.ascontiguousarray(np.asarray(Wo)[:, hs].T).astype(bf),
            "biases": biases,
            "cos_t": cos_t, "sin_t": sin_t,
        })
    return in_maps


_NC_CACHE = {}


def _get_nc():
    if "nc" not in _NC_CACHE:
        nc = build_nc()
        nc.finalize()
        _NC_CACHE["nc"] = nc
    return _NC_CACHE["nc"]


def kernel(q, k, v, Wq, bq, Wk, bk, Wv, bv, Wo, bo):
    assert q.shape == (B, L, D) and k.shape == (B, L, D) and v.shape == (B, L, D)
    in_maps = host_in_maps(q, k, v, Wq, bq, Wk, bk, Wv, bv, Wo)
    nc = _get_nc()
    res = run_bass_kernel_spmd(nc, in_maps, list(range(NCORES)))
    out = np.zeros((TOK, D), np.float64)
    for r in res.results:
        out += r["outp"].astype(np.float64)
    out += np.asarray(bo, np.float64)[None, :]
    return out.astype(np.float32).reshape(B, L, D)


# revision 58
# speedup vs baseline: 1.2122x; 1.0285x over previous
"""Trainium2 Bass kernel for MultiHeadAttention with RoPE.

Problem: B=2, L=2048, d_model=1024, 16 heads, d_k=64, fp32 in/out.

Sharding (8 cores): tensor-parallel over heads — core c owns heads
{2c, 2c+1}, i.e. a 128-wide slice of the projection output dims.  Every
core reads the full q/k/v activations (transposed + bf16 on host), its
own 128-row slice of Wq/Wk/Wv (pre-transposed; Wq/bq pre-scaled by
1/sqrt(dk)) and the matching 128 columns of Wo.  Each core computes its
heads' attention output and a partial d_model output projection; the
host sums the 8 partials and adds bo.

Per-core pipeline (bf16 matmuls, fp32 PSUM):
  1. QKV projections [128 pd, 1024 tok] halves; bias-add + bf16 evict on
     DVE; RoPE via partition-swap DMAs + 3 DVE ops (sign folded in sin
     table, 1/sqrt(dk) folded into Wq).
  2. V-heads transposed to [kt, dim] layout by ONE dma_start_transpose
     per (batch, head) into a 65-wide-stride "vaug" buffer whose 65th
     column is ones.
  3. Scores: per 128-kt tile, the two heads run CONCURRENTLY on the PE
     as K=64 row-tiles (tile_position (0,0) / (64,0) auto-derived).
  4. exp on ScalarE ([128, 1024] per kt tile covering both heads).
  5. ctx: lhsT = vaug [128 kt, 65] per head; row 64 accumulates the
     softmax denominator for free (M=65 stationary).
  6. normalize: DVE copy of cp, denominator row -> partition 0 via DMA,
     reciprocal + GpSimd broadcast + DVE muls; h1 ctx shifted to
     partitions 64:127 by a small DMA.
  7. out_proj [tok, 1024] = ctx (stationary) @ WoT slice.
Emission interleaves next-batch projections and out_proj matmuls into
the (ScalarE-bound) attention loops as "fillers" so the PE never idles
long and HAM stays at full clock.  All activation buffers are
per-batch tiles so filler writes never create false WAR dependencies
against the running attention.
"""

import collections
import numpy as np
import ml_dtypes

import concourse.bass as bass
import concourse.mybir as mybir
import concourse.tile as tile
from concourse import bacc
from concourse.bass_utils import run_bass_kernel_spmd

BF = mybir.dt.bfloat16
F32 = mybir.dt.float32
AF = mybir.ActivationFunctionType

NCORES = 8
B = 2
L = 2048
D = 1024          # d_model
H = 16            # heads
DK = 64           # head dim
HPC = H // NCORES  # heads per core = 2
PD = HPC * DK      # projection dims per core = 128
TOK = B * L        # 4096 tokens
P = 128
NKT = L // P       # 16 kt tiles per batch
NQB = 4            # 512-token q blocks per batch

ROPE_BASE = 10000.0


def build_nc(debug_dumps=False):
    """Build the single-core Bass program (SPMD: same program, per-core data)."""
    from contextlib import ExitStack

    nc = bacc.Bacc("TRN2", target_bir_lowering=False, debug=False)
    dbg = {}
    if debug_dumps:
        for nm, shp, dt in [
            ("dbg_qq", [P, L], BF), ("dbg_kk", [P, L], BF),
            ("dbg_vaug0", [P, NKT * P], BF), ("dbg_vaug1", [P, NKT * P], BF),
            ("dbg_exp", [P, 1024], BF), ("dbg_cps", [65, 1024], F32),
            ("dbg_rec", [1, 1024], F32), ("dbg_ctx", [P, L], BF),
        ]:
            dbg[nm] = nc.dram_tensor(nm, shp, dt, kind="ExternalOutput").ap()

    # ---- DRAM I/O ----
    # Host pre-arranges everything partition-contiguous so each DMA is 128
    # descriptors of big contiguous chunks (dispatch cost ~ descriptor count).
    # xH[p, (b, half, a, t)]: token t of half `half` of batch b, dim a*128+p.
    qH = nc.dram_tensor("qH", [P, B * 2 * 8 * 1024], BF, kind="ExternalInput").ap()
    kH = nc.dram_tensor("kH", [P, B * 2 * 8 * 1024], BF, kind="ExternalInput").ap()
    vH = nc.dram_tensor("vH", [P, B * 2 * 8 * 1024], BF, kind="ExternalInput").ap()
    # wH[p, (a, m)]: weight row a*128+p, output dim m.
    wqH = nc.dram_tensor("wqH", [P, 8 * P], BF, kind="ExternalInput").ap()
    wkH = nc.dram_tensor("wkH", [P, 8 * P], BF, kind="ExternalInput").ap()
    wvH = nc.dram_tensor("wvH", [P, 8 * P], BF, kind="ExternalInput").ap()
    woT = nc.dram_tensor("woT", [PD, D], BF, kind="ExternalInput").ap()
    bias_d = nc.dram_tensor("biases", [PD, 3], F32, kind="ExternalInput").ap()
    cos_d = nc.dram_tensor("cos_t", [P, L], BF, kind="ExternalInput").ap()
    sin_d = nc.dram_tensor("sin_t", [P, L], BF, kind="ExternalInput").ap()
    outp = nc.dram_tensor("outp", [TOK, D], BF, kind="ExternalOutput").ap()

    xT = {"q": qH, "k": kH, "v": vH}

    with tile.TileContext(nc) as tc, ExitStack() as ctx:
        const = ctx.enter_context(tc.tile_pool(name="const", bufs=1))
        persist = ctx.enter_context(tc.tile_pool(name="persist", bufs=1))
        stage = ctx.enter_context(tc.tile_pool(name="stage", bufs=5))
        raws = ctx.enter_context(tc.tile_pool(name="raws", bufs=2))
        rots = ctx.enter_context(tc.tile_pool(name="rots", bufs=2))
        expp = ctx.enter_context(tc.tile_pool(name="expp", bufs=3))
        outs = ctx.enter_context(tc.tile_pool(name="outs", bufs=3))
        smalls = ctx.enter_context(tc.tile_pool(name="smalls", bufs=2))
        h1p = ctx.enter_context(tc.tile_pool(name="h1p", bufs=2))
        # PSUM: scores 2 tiles x 2 banks + ctx 2 banks + proj/out 2 banks = 8
        scp = ctx.enter_context(tc.tile_pool(name="scp", bufs=2, space="PSUM"))
        cpp = ctx.enter_context(tc.tile_pool(name="cpp", bufs=1, space="PSUM"))
        pop = ctx.enter_context(tc.tile_pool(name="pop", bufs=1, space="PSUM"))

        # ---- constants (declared here, loads emitted inside the barrier
        # chain below in first-use order) ----
        def load_w(name, w_d):
            w_sb = const.tile([P, 8 * P], BF, name=name)
            nc.sync.dma_start(w_sb[:], w_d[:])
            return w_sb

        bias_sb3 = const.tile([P, 3], F32, name="bias_sb3")
        bq_sb = bias_sb3[:, 0:1]
        bk_sb = bias_sb3[:, 1:2]
        bv_sb = bias_sb3[:, 2:3]
        cos_sb = const.tile([P, L], BF, name="cos_sb")
        sin_sb = const.tile([P, L], BF, name="sin_sb")

        # per-(batch, token-half) persistent activations [128 dims, 1024 tok]
        # (split so scores never wait on the other half's rope chain)
        qq_h = [[persist.tile([P, 1024], BF, name=f"qq{b}_{hf}")
                 for hf in range(2)] for b in range(B)]
        kk_h = [[persist.tile([P, 1024], BF, name=f"kk{b}_{hf}")
                 for hf in range(2)] for b in range(B)]
        ctx_b = [persist.tile([P, L], BF, name=f"ctx{b}") for b in range(B)]
        # vaug[b][h]: 16 slots of [128 kt, 128]; cols 0:64 = v dims
        # (t-major: slot t partition p holds token t*128+p), cols 64:128 all
        # ones — so the ctx matmul replicates the softmax denominator into
        # cp rows 64:128 (no partition broadcast needed to normalize).
        vaug = [[persist.tile([P, NKT * P], BF, name=f"vaug_{b}_{h}")
                 for h in range(2)] for b in range(B)]
        for b in range(B):
            for h in range(2):
                va = vaug[b][h].rearrange("p (t u) -> p t u", u=P)
                nc.vector.memset(va[:, :, DK:P], 1.0)

        # ---------- filler machinery ----------
        fillers = collections.deque()

        def fill(budget):
            while fillers and budget > 0:
                cost, fn = fillers.popleft()
                fn()
                budget -= cost

        def flush():
            while fillers:
                fillers.popleft()[1]()

        # ---------- phase helpers ----------
        def load_half(which, b, half, eng=None):
            """One 2MB DMA, per-partition contiguous: a 1024-token half."""
            xt = stage.tile([P, 8 * 1024], BF, name="xstage", tag="stage")
            j = (b * 2 + half) * 8192
            (eng or nc.sync).dma_start(xt[:], xT[which][:, j:j + 8192])
            return xt

        def proj_units(which, b, w_sb, bias_sb, dst_sb=None, vh_cb=None,
                       preloaded=None, halves=(0, 1), shared=None,
                       split_muls=False):
            """Filler units projecting batch b (chosen 1024-token halves).

            dst_sb given -> rope into it (q/k).  vh_cb given -> v path:
            evict to a fresh vh tile, call vh_cb(vh_tile) when done.
            preloaded: stage tiles already loaded.  Pass the same `shared`
            dict when splitting one projection across two calls.
            """
            units = []
            if shared is None:
                shared = {}
            if preloaded is not None:
                shared[("x", 0)], shared[("x", 1)] = preloaded

            def alloc_pp(half):
                def go():
                    shared[("pp", half)] = pop.tile(
                        [P, 1024], F32, name="pp", tag="pp")
                return go

            def mm_kc(half, kc):
                def go():
                    xt = shared[("x", half)].rearrange("p (a t) -> p a t", a=8)
                    pp = shared[("pp", half)]
                    for nb in range(2):
                        nc.tensor.matmul(
                            pp[:, nb * 512:(nb + 1) * 512],
                            lhsT=w_sb[:, kc * P:(kc + 1) * P],
                            rhs=xt[:, kc, nb * 512:(nb + 1) * 512],
                            start=(kc == 0), stop=(kc == 7),
                        )
                return go

            def evict_rot(half):
                def go():
                    pp = shared[("pp", half)]
                    raw = raws.tile([P, 1024], BF, name="raw", tag="raw")
                    nc.vector.tensor_scalar_add(raw[:], pp[:], bias_sb[:])
                    rot = rots.tile([P, 1024], BF, name="rot", tag="rot")
                    for h in range(2):
                        r0 = h * DK
                        nc.sync.dma_start(rot[r0:r0 + 32, :],
                                          raw[r0 + 32:r0 + 64, :])
                        nc.sync.dma_start(rot[r0 + 32:r0 + 64, :],
                                          raw[r0:r0 + 32, :])
                    shared[("rr", half)] = (raw, rot)
                return go

            def rope_mul(half):
                def go():
                    raw, rot = shared[("rr", half)]
                    cs = slice(half * 1024, (half + 1) * 1024)
                    nc.vector.tensor_mul(raw[:], raw[:], cos_sb[:, cs])
                    nc.vector.tensor_mul(rot[:], rot[:], sin_sb[:, cs])
                    nc.vector.tensor_add(dst_sb[half][:], raw[:], rot[:])
                return go

            def evict_v(half):
                def go():
                    pp = shared[("pp", half)]
                    if "vh" not in shared:
                        shared["vh"] = raws.tile(
                            [P, L], BF, name="vhs", tag="vraw", bufs=2)
                    vh = shared["vh"]
                    nc.vector.tensor_scalar_add(
                        vh[:, half * 1024:(half + 1) * 1024], pp[:], bias_sb[:])
                    if half == 1:
                        vh_cb(vh)
                return go

            # rope_mul units must be sequenced well AFTER their evict+rot
            # so the partition-swap DMA transfers overlap later matmuls
            # instead of stalling the in-order DVE queue.
            muls = []
            for half in halves:
                units.append((0, alloc_pp(half)))
                for kc in range(8):
                    units.append((2, mm_kc(half, kc)))
                if dst_sb is not None:
                    units.append((2, evict_rot(half)))
                    muls.append((2, rope_mul(half)))
                else:
                    units.append((2, evict_v(half)))
            if split_muls:
                return units, muls
            units.extend(muls)
            return units

        def vaug_transpose(b):
            # HW xbar transpose requires a contiguous destination; land in
            # per-head vt tiles then DVE-copy into the vaug slots.  Batch 0
            # uses the (pre-exp idle) scalar hwdge queue so the transpose's
            # queue-drain never blocks the rope DMAs behind it on sync.
            eng = nc.scalar if b == 0 else nc.sync
            def go(vh):
                for h in range(2):
                    vt = rots.tile([P, 1024], BF, name="vt", tag=f"vt{h}",
                                   bufs=2)
                    eng.dma_start_transpose(
                        vt.rearrange("p (t u) -> p t u", u=64),
                        vh[h * DK:(h + 1) * DK, :])
                    va = vaug[b][h].rearrange(
                        "p (t u) -> p t u", u=P)[:, :, 0:DK]
                    nc.vector.tensor_copy(
                        va, vt.rearrange("p (t u) -> p t u", u=64))
            return go

        def attention(b, qb, budget=3):
            """512 q tokens; 16 kt tiles; 2 heads row-tiled on the PE."""
            qsl = slice((qb % 2) * 512, (qb % 2) * 512 + 512)
            qq = qq_h[b][qb // 2]
            q0 = qb * 512
            cp = cpp.tile([P, 1024], F32, name="cp", tag="cp")
            ex_prev = None
            for kt in range(NKT + 1):
                ex_cur = None
                if kt < NKT:
                    kk = kk_h[b][kt // 8]
                    ksl = slice((kt % 8) * P, (kt % 8) * P + P)
                    sc = scp.tile([P, 1024], F32, name="sc", tag="sc")
                    nc.tensor.matmul(
                        sc[:, 0:512],
                        lhsT=kk[0:DK, ksl],
                        rhs=qq[0:DK, qsl],
                        start=True, stop=True, skip_group_check=True)
                    nc.tensor.matmul(
                        sc[:, 512:1024],
                        lhsT=kk[DK:P, ksl],
                        rhs=qq[DK:P, qsl],
                        start=True, stop=True, skip_group_check=True)
                    ex_cur = expp.tile([P, 1024], BF, name="ex", tag="ex")
                    nc.scalar.activation(ex_cur[:], sc[:], AF.Exp)
                    if debug_dumps and b == 0 and qb == 0 and kt == 0:
                        nc.sync.dma_start(dbg["dbg_exp"][:], ex_cur[:])
                if kt >= 1:
                    c = kt - 1
                    for h in range(2):
                        nc.tensor.matmul(
                            cp[:, h * 512:(h + 1) * 512],
                            lhsT=vaug[b][h][:, c * P:(c + 1) * P],
                            rhs=ex_prev[:, h * 512:(h + 1) * 512],
                            start=(c == 0), stop=(c == NKT - 1),
                            skip_group_check=True)
                ex_prev = ex_cur
                fill(budget)
            # normalize: cp rows 64:128 all hold the denominator; copy cp
            # out (frees the psum), DMA the denominator rows to partition 0,
            # reciprocal, scale both heads.
            cps = smalls.tile([P, 1024], F32, name="cps", tag="cps")
            nc.vector.tensor_copy(cps[:], cp[:])
            den = smalls.tile([DK, 1024], F32, name="den", tag="den", bufs=1)
            nc.sync.dma_start(den[:], cps[DK:P, :])
            rec = smalls.tile([DK, 1024], F32, name="rec", tag="rec", bufs=1)
            nc.vector.reciprocal_approx_fast(rec[:], den[:])
            if debug_dumps and b == 0 and qb == 0:
                nc.sync.dma_start(dbg["dbg_cps"][:], cps[0:65, :])
                nc.sync.dma_start(dbg["dbg_rec"][:], rec[0:1, :])
            nc.vector.tensor_mul(
                ctx_b[b][0:DK, q0:q0 + 512], cps[0:DK, 0:512], rec[:, 0:512])
            h1s = h1p.tile([DK, 512], BF, name="h1s", tag="h1s")
            nc.vector.tensor_mul(h1s[:], cps[0:DK, 512:1024], rec[:, 512:1024])
            nc.sync.dma_start(ctx_b[b][DK:P, q0:q0 + 512], h1s[:])

        def out_units(b, qb, tail=False):
            units = []

            def po_unit(tb):
                def go():
                    t0 = qb * 512 + tb * P
                    # in the end-of-kernel tail, alternate PSUM pools so
                    # consecutive units pipeline instead of serializing on
                    # the single pop buffer (scores pool is idle by then)
                    if tail and tb % 2:
                        po = scp.tile([P, D], F32, name="sc", tag="sc")
                    else:
                        po = pop.tile([P, D], F32, name="po", tag="pp")
                    for nb in range(2):
                        nc.tensor.matmul(
                            po[:, nb * 512:(nb + 1) * 512],
                            lhsT=ctx_b[b][:, t0:t0 + P],
                            rhs=wo_sb[:, nb * 512:(nb + 1) * 512],
                            start=True, stop=True, skip_group_check=True)
                    ob = outs.tile([P, D], BF, name="ob", tag="ob")
                    nc.vector.tensor_copy(ob[:], po[:])
                    nc.sync.dma_start(outp[b * L + t0:b * L + t0 + P, :], ob[:])
                return go

            for tb in range(4):
                units.append((2, po_unit(tb)))
            return units

        def load_bar(tile):
            """Tiny DMA reading `tile`: holds its queue until tile's load
            transfer completes, serializing big transfers so they finish
            in consumption order instead of round-robin sharing."""
            dm = smalls.tile([1, 2], BF, name="bar", tag="bar", bufs=2)
            nc.sync.dma_start(dm[:], tile[0:1, 0:2])

        # ---------- program ----------
        # Batch-0 inputs prefetched in consumption-criticality order and
        # SERIALIZED via load barriers: v first (the vaug chain gates the
        # first ctx matmul), then the h0 halves of k and q (which gate the
        # first scores), then k-h1 / q-h1 for the filler projections.
        # v first (gates the first ctx), then wv; k-h0/q-h0 (gate the first
        # scores) with their weights and the rope tables slotted into the
        # barrier gaps; k-h1/q-h1 last.
        v0a = load_half("v", 0, 0)
        v0b = load_half("v", 0, 1)
        v0_x = (v0a, v0b)
        load_bar(v0a)
        wv_sb = load_w("wv_sb", wvH)
        nc.sync.dma_start(bias_sb3[:], bias_d[:])
        k0a = load_half("k", 0, 0)
        load_bar(v0b)
        wk_sb = load_w("wk_sb", wkH)
        q0a = load_half("q", 0, 0)
        wq_sb = load_w("wq_sb", wqH)
        load_bar(k0a)
        nc.sync.dma_start(cos_sb[:], cos_d[:])
        nc.sync.dma_start(sin_sb[:], sin_d[:])
        k0b = load_half("k", 0, 1)
        load_bar(q0a)
        q0b = load_half("q", 0, 1)
        wo_sb = const.tile([P, D], BF)
        nc.sync.dma_start(wo_sb[:], woT[:])
        k0_x = (k0a, k0b)
        q0_x = (q0a, q0b)

        for _, fn in proj_units("v", 0, wv_sb, bv_sb, vh_cb=vaug_transpose(0),
                                preloaded=v0_x):
            fn()
        ksh, qsh = {}, {}
        k_main, k_muls = proj_units("k", 0, wk_sb, bk_sb, dst_sb=kk_h[0],
                                    preloaded=k0_x, halves=(0,), shared=ksh,
                                    split_muls=True)
        q_main, q_muls = proj_units("q", 0, wq_sb, bq_sb, dst_sb=qq_h[0],
                                    preloaded=q0_x, halves=(0,), shared=qsh,
                                    split_muls=True)
        # k-ev -> q-proj -> k-mul -> q-mul: the rot transfers overlap the
        # other projection's matmuls instead of stalling the DVE queue.
        for _, fn in k_main + q_main + k_muls + q_muls:
            fn()
        # second halves become the first attention fillers
        kh1_m, kh1_x = proj_units("k", 0, wk_sb, bk_sb, dst_sb=kk_h[0],
                                  halves=(1,), shared=ksh, split_muls=True)
        qh1_m, qh1_x = proj_units("q", 0, wq_sb, bq_sb, dst_sb=qq_h[0],
                                  halves=(1,), shared=qsh, split_muls=True)
        # k-h1's rope mul MUST be emitted before attention(0, qb0) reaches
        # kt tile 8 (which reads kk_h[0][1]); budget 4 drains it by ~kt 5.
        fillers.extend(kh1_m + kh1_x + qh1_m + qh1_x)
        # batch-1 k prefetch rides the tail of the batch-0 load stream
        xk1 = (load_half("k", 1, 0), load_half("k", 1, 1))

        # batch 0 attention; feed batch-1 projections + batch-0 out_proj
        # into the scalar-bound loop as fillers (batch-1 input DMAs are
        # dispatched inline at block boundaries for transfer lead time).
        for qb in range(NQB):
            attention(0, qb, budget=4 if qb == 0 else 3)
            fillers.extend(out_units(0, qb))
            if qb == 0:
                xv1 = (load_half("v", 1, 0), load_half("v", 1, 1))
                fillers.extend(proj_units("k", 1, wk_sb, bk_sb,
                                          dst_sb=kk_h[1], preloaded=xk1))
                fillers.extend(proj_units("v", 1, wv_sb, bv_sb,
                                          vh_cb=vaug_transpose(1),
                                          preloaded=xv1))
            elif qb == 1:
                xq1 = (load_half("q", 1, 0), load_half("q", 1, 1))
                fillers.extend(proj_units("q", 1, wq_sb, bq_sb,
                                          dst_sb=qq_h[1], preloaded=xq1))
        flush()  # b1 attention depends on b1 projections: drain first

        for qb in range(NQB):
            attention(1, qb)
            fillers.extend(out_units(1, qb, tail=(qb == NQB - 1)))
        flush()

        if debug_dumps:
            for hf in range(2):
                nc.sync.dma_start(
                    dbg["dbg_qq"][:, hf * 1024:(hf + 1) * 1024], qq_h[0][hf][:])
                nc.sync.dma_start(
                    dbg["dbg_kk"][:, hf * 1024:(hf + 1) * 1024], kk_h[0][hf][:])
            nc.sync.dma_start(dbg["dbg_vaug0"][:], vaug[0][0][:])
            nc.sync.dma_start(dbg["dbg_vaug1"][:], vaug[0][1][:])
            nc.sync.dma_start(dbg["dbg_ctx"][:], ctx_b[0][:])

    return nc


def _rope_tables():
    """Host-built RoPE tables [d, t], 2 heads stacked, sign-folded sin."""
    inv_freq = 1.0 / (ROPE_BASE ** (np.arange(0, DK, 2, dtype=np.float64) / DK))
    t = np.arange(L, dtype=np.float64)
    ang = np.outer(t, inv_freq)               # [L, 32]
    emb = np.concatenate([ang, ang], axis=1)  # [L, 64]
    cos = np.cos(emb).T.astype(np.float32)    # [64, L]
    sin = np.sin(emb).T.astype(np.float32)
    sin_folded = sin.copy()
    sin_folded[:32] *= -1.0
    bf = ml_dtypes.bfloat16
    cos2 = np.concatenate([cos, cos], axis=0)                # [128, L]
    sin2 = np.concatenate([sin_folded, sin_folded], axis=0)  # [128, L]
    return cos2.astype(bf), sin2.astype(bf)


def _xh(x):
    """[B, L, D] -> [128, B*2*8*1024]: xH[p, (b, hf, a, t)] = x[b, hf*1024+t,
    a*128+p] — every (b, half) load is per-partition contiguous."""
    bf = ml_dtypes.bfloat16
    xr = np.asarray(x).reshape(B, 2, 1024, 8, P).transpose(4, 0, 1, 3, 2)
    return np.ascontiguousarray(xr.reshape(P, B * 2 * 8 * 1024)).astype(bf)


def _wh(w_slice):
    """[PD, D] weight slice -> [128, 8*128]: wH[p, (a, m)] = W.T[a*128+p, m]."""
    bf = ml_dtypes.bfloat16
    wr = np.ascontiguousarray(w_slice.T).reshape(8, P, PD).transpose(1, 0, 2)
    return np.ascontiguousarray(wr.reshape(P, 8 * PD)).astype(bf)


def host_in_maps(q, k, v, Wq, bq, Wk, bk, Wv, bv, Wo):
    """Per-core input maps (the 1/sqrt(dk) scale is folded into Wq/bq)."""
    bf = ml_dtypes.bfloat16
    qh, kh, vh = _xh(q), _xh(k), _xh(v)
    cos_t, sin_t = _rope_tables()
    scale = 1.0 / np.sqrt(DK)
    in_maps = []
    for c in range(NCORES):
        hs = slice(c * PD, (c + 1) * PD)
        biases = np.stack([
            np.asarray(bq)[hs] * scale, np.asarray(bk)[hs],
            np.asarray(bv)[hs]], axis=1).astype(np.float32)
        in_maps.append({
            "qH": qh, "kH": kh, "vH": vh,
            "wqH": _wh(np.asarray(Wq)[hs, :] * scale),
            "wkH": _wh(np.asarray(Wk)[hs, :]),
            "wvH": _wh(np.asarray(Wv)[hs, :]),
            "woT": np.ascontiguousarray(np.asarray(Wo)[:, hs].T).astype(bf),
            "biases": biases,
            "cos_t": cos_t, "sin_t": sin_t,
        })
    return in_maps


_NC_CACHE = {}


def _get_nc():
    if "nc" not in _NC_CACHE:
        nc = build_nc()
        nc.finalize()
        _NC_CACHE["nc"] = nc
    return _NC_CACHE["nc"]


def kernel(q, k, v, Wq, bq, Wk, bk, Wv, bv, Wo, bo):
    assert q.shape == (B, L, D) and k.shape == (B, L, D) and v.shape == (B, L, D)
    in_maps = host_in_maps(q, k, v, Wq, bq, Wk, bk, Wv, bv, Wo)
    nc = _get_nc()
    res = run_bass_kernel_spmd(nc, in_maps, list(range(NCORES)))
    out = np.zeros((TOK, D), np.float64)
    for r in res.results:
        out += r["outp"].astype(np.float64)
    out += np.asarray(bo, np.float64)[None, :]
    return out.astype(np.float32).reshape(B, L, D)
